# revision 1
# baseline (speedup 1.0000x reference)
"""CapsuleConv2d (3-iteration dynamic routing) Bass kernel for 8 TRN2 cores.

Strategy (data-parallel over batch, 2 images per core):
  - priors[l, ij, o, u, f] computed by PE per 128-location tile:
    stationary = padded-x window [32=(f,d), 128 locs], moving = structured
    weight constants [32, (o,u,f)] per kernel tap ij.  s0 = 0.25*sum_r priors
    accumulated by PE in the same pass.
  - routing in "natural" layout (locations on partitions): DVE does the
    broadcast-multiplies + segmented reduces, ACT does exp/square/sqrt and
    PSUM->SBUF copies, PE transposes the final [128 locs, 32 ch] result for
    channel-major DMA out.
"""
import numpy as np

import concourse.bass as bass
import concourse.bacc as bacc
import concourse.tile as tile
import concourse.mybir as mybir
import concourse.bass_utils as bass_utils

# All ACT functions we use (Exp, Ln, Square, Copy, ...) live together in the
# "natural_log_exp_and_others" table set, but bacc's table-load pass picks a
# per-function set greedily (Ln -> natural_log, Exp -> exp_and_others),
# thrashing ~2.7us table loads between them.  Restrict Exp/Ln to the combined
# set so a single load covers the whole kernel.
_orig_get_tables = bacc.get_activation_tables
_AFT = mybir.ActivationFunctionType


def _patched_get_tables(arch):
    tables = dict(_orig_get_tables(arch))
    for name, funcs in tables.items():
        if name != "natural_log_exp_and_others":
            tables[name] = funcs - {_AFT.Exp, _AFT.Ln}
    return tables


bacc.get_activation_tables = _patched_get_tables

# ---- problem constants (hardcoded; must match setup_inputs) ----
O, F, U, D = 4, 4, 8, 8
KH = KW = 3
NIJ = KH * KW
H = W = 64
C = 32
N_FULL = 16
N_CORES = 8
IMG_PER_CORE = N_FULL // N_CORES
HP, WP = H + 2, W + 2              # padded input
LT_ROWS = 2                        # output rows per 128-loc tile
NLT = H // LT_ROWS                 # 32 loc-tiles per image
ST_LT = 4                          # loc-tiles per super-tile (512 locs)
NST = NLT // ST_LT                 # 8 super-tiles per image
PB = 2                             # super-tiles batched per routing pass
PLT = PB * ST_LT                   # loc-tiles per routing pass (8)
EPS = 1e-12

f32 = mybir.dt.float32
AL = mybir.AluOpType
AF = mybir.ActivationFunctionType
AX = mybir.AxisListType

_COMPILED = None


def _build(dump=False, repeat=1):
    nc = bacc.Bacc("TRN2", target_bir_lowering=False, debug=False)

    dbg = {}
    if dump:
        for name, shape in [("dbg_P", [128, ST_LT * 1152]),
                            ("dbg_s0", [128, ST_LT * 32]),
                            ("dbg_v0", [128, ST_LT * 32]),
                            ("dbg_b1", [128, ST_LT * 144]),
                            ("dbg_E1", [128, ST_LT * 144]),
                            ("dbg_s1", [128, ST_LT * 32])]:
            dbg[name] = nc.dram_tensor(name, shape, f32,
                                       kind="ExternalOutput").ap()

    xin_d = nc.dram_tensor("xin", [IMG_PER_CORE, C, H * W], f32,
                           kind="ExternalInput").ap()
    wmov_d = nc.dram_tensor("wmov", [C, NIJ * 128], f32,
                            kind="ExternalInput").ap()
    wsum_d = nc.dram_tensor("wsum", [C, NIJ * 32], f32,
                            kind="ExternalInput").ap()
    ident_d = nc.dram_tensor("ident", [128, 128], f32,
                             kind="ExternalInput").ap()
    out_d = nc.dram_tensor("out", [IMG_PER_CORE, C, H * W], f32,
                           kind="ExternalOutput").ap()

    with tile.TileContext(nc) as tc:
        with tc.tile_pool(name="const", bufs=1) as cpool, \
             tc.tile_pool(name="xpad", bufs=1) as xpool, \
             tc.tile_pool(name="stage", bufs=1) as spool, \
             tc.tile_pool(name="pst", bufs=4) as ppool, \
             tc.tile_pool(name="gh", bufs=2) as ghpool, \
             tc.tile_pool(name="small", bufs=2) as smpool, \
             tc.tile_pool(name="ppri", bufs=2, space="PSUM") as ppri, \
             tc.tile_pool(name="ps0", bufs=1, space="PSUM") as ps0, \
             tc.tile_pool(name="ptp", bufs=1, space="PSUM") as ptp:

            wmov_s = cpool.tile([C, NIJ * 128], f32, tag="wmov")
            wsum_s = cpool.tile([C, NIJ * 32], f32, tag="wsum")
            ident_s = cpool.tile([128, 128], f32, tag="ident")
            eps_s = cpool.tile([128, 1], f32, tag="eps")
            nc.sync.dma_start(wmov_s[:], wmov_d[:])
            nc.sync.dma_start(wsum_s[:], wsum_d[:])
            nc.sync.dma_start(ident_s[:], ident_d[:])
            nc.gpsimd.memset(eps_s[:], EPS)

            for img in range(IMG_PER_CORE):
                # one shared xpad slot: image n+1's fill overlaps image n's
                # routing tail (P-production finishes ~100us early)
                xp = xpool.tile([C, HP * WP], f32, tag="xpad")
                nc.gpsimd.memset(xp[:], 0.0)
                xv = xp[:].rearrange("p (h w) -> p h w", h=HP, w=WP)
                nc.sync.dma_start(
                    xv[:, 1:1 + H, 1:1 + W],
                    xin_d[img].rearrange("p (h w) -> p h w", h=H, w=W))
                xpads = {img: xv}
                stage = spool.tile([C, H * W], f32, tag="stage")
                for pr_rep in range((NST // PB) * repeat):
                    pr = pr_rep % (NST // PB)
                    # produce priors for PB super-tiles; small per-location
                    # tensors are batched across the pair to amortize DVE
                    # per-instruction overhead.
                    P_sts = []
                    s0_st = smpool.tile([128, PLT * 32], f32, tag="s0")
                    for half in range(PB):
                        st = pr * PB + half
                        P_st = ppool.tile([128, ST_LT * 1152], f32, tag="P")
                        P_sts.append(P_st)
                        for lt in range(ST_LT):
                            r0 = (st * ST_LT + lt) * LT_ROWS
                            glt = half * ST_LT + lt
                            pp = ppri.tile([128, 1152], f32, tag="ppri")
                            s0p = ps0.tile([128, 32], f32, tag="s0p")
                            for ij in range(NIJ):
                                i, j = ij // KW, ij % KW
                                for r in range(LT_ROWS):
                                    xw = xpads[img][:, r0 + i + r, j:j + W]
                                    prow = slice(r * W, (r + 1) * W)
                                    nc.tensor.matmul(
                                        pp[prow, ij * 128:(ij + 1) * 128],
                                        xw,
                                        wmov_s[:, ij * 128:(ij + 1) * 128],
                                        start=True, stop=True)
                                    nc.tensor.matmul(
                                        s0p[prow], xw,
                                        wsum_s[:, ij * 32:(ij + 1) * 32],
                                        start=(ij == 0),
                                        stop=(ij == NIJ - 1))
                            nc.scalar.copy(
                                P_st[:, lt * 1152:(lt + 1) * 1152], pp[:])
                            nc.scalar.copy(
                                s0_st[:, glt * 32:(glt + 1) * 32], s0p[:])

                    # ------- routing on this super-tile pair -------
                    def P5(half, lt):
                        return P_sts[half][:, lt * 1152:(lt + 1) *
                                           1152].rearrange(
                            "p (ij o u f) -> p ij o u f", ij=NIJ, o=O, u=U,
                            f=F)

                    def squash(s_st, tagp, newton=True):
                        # s_st: [128, (lt, o, u)]; returns v [128, (lt,o,u)]
                        # sqrt & reciprocals go through the ACT exp/ln
                        # tables (single table set, no DVE reciprocal);
                        # one Newton step restores sqrt to fp32 accuracy.
                        # newton=False skips it where the ~5e-6 table error
                        # is not amplified (the final squash: error passes
                        # straight to the output instead of through the
                        # routing logits).
                        sq = smpool.tile([128, PLT * 32], f32,
                                         tag=f"sq{tagp}")
                        nc.scalar.activation(sq[:], s_st[:], AF.Square)
                        n2 = smpool.tile([128, PLT * O], f32,
                                         tag=f"n2{tagp}")
                        nc.vector.tensor_reduce(
                            n2[:],
                            sq[:].rearrange("p (lt o u) -> p lt o u",
                                            lt=PLT, o=O, u=U),
                            AX.X, AL.add)
                        Ltile = smpool.tile([128, PLT * O], f32,
                                            tag=f"L{tagp}")
                        nc.scalar.activation(Ltile[:], n2[:], AF.Ln,
                                             bias=eps_s[:])
                        t_ = smpool.tile([128, PLT * O], f32,
                                         tag=f"t{tagp}")
                        nc.scalar.activation(t_[:], Ltile[:], AF.Exp,
                                             scale=0.5)
                        if newton:
                            r5 = smpool.tile([128, PLT * O], f32,
                                             tag=f"r5{tagp}")
                            # true Newton needs 1/t of the current t —
                            # exact DVE reciprocal (exp(-ln t) tables are
                            # ~1e-5 off)
                            nc.vector.reciprocal(r5[:], t_[:])
                            xr = smpool.tile([128, PLT * O], f32,
                                             tag=f"xr{tagp}")
                            # xr = n2 * (0.5/t)  (eps negligible: fi -> 0
                            # as n2 -> 0 regardless)
                            nc.vector.scalar_tensor_tensor(
                                xr[:], r5[:], 0.5, n2[:], AL.mult, AL.mult)
                            # t = 0.5*t + xr   (Newton)
                            nc.vector.scalar_tensor_tensor(
                                t_[:], t_[:], 0.5, xr[:], AL.mult, AL.add)
                        # w = (1+n2)*t;  fi = n2 / w
                        pw = smpool.tile([128, PLT * O], f32,
                                         tag=f"pw{tagp}")
                        nc.vector.scalar_tensor_tensor(
                            pw[:], n2[:], 1.0, t_[:], AL.add, AL.mult)
                        rw = smpool.tile([128, PLT * O], f32,
                                         tag=f"rw{tagp}")
                        nc.vector.reciprocal(rw[:], pw[:])
                        fi = smpool.tile([128, PLT * O], f32,
                                         tag=f"fi{tagp}")
                        nc.vector.tensor_tensor(fi[:], n2[:], rw[:], AL.mult)
                        v = smpool.tile([128, PLT * 32], f32,
                                        tag=f"v{tagp}")
                        fib = fi[:].rearrange("p (lt o) -> p lt o",
                                              lt=PLT).unsqueeze(3)
                        nc.vector.tensor_tensor(
                            v[:].rearrange("p (lt o u) -> p lt o u",
                                           lt=PLT, o=O, u=U),
                            s_st[:].rearrange("p (lt o u) -> p lt o u",
                                              lt=PLT, o=O, u=U),
                            fib.broadcast_to((128, PLT, O, U)), AL.mult)
                        return v

                    is_dbg = dump and img == 0 and pr == 0
                    if is_dbg:
                        nc.sync.dma_start(dbg["dbg_P"][:], P_sts[0][:])
                        nc.sync.dma_start(dbg["dbg_s0"][:],
                                          s0_st[:, :ST_LT * 32])

                    v = squash(s0_st, "0")
                    if is_dbg:
                        nc.sync.dma_start(dbg["dbg_v0"][:],
                                          v[:, :ST_LT * 32])

                    # b1[l, (half, lt, ij, o, f)] = sum_u P * v0
                    b_st = smpool.tile([128, PLT * 144], f32, tag="b")
                    hred = smpool.tile([128, PLT * 144], f32, tag="hred")
                    for it in range(3):
                        if it > 0:
                            # E = exp(b); Z = sum_o E; E' = E / Z
                            E = smpool.tile([128, PLT * 144], f32, tag="E")
                            nc.scalar.activation(E[:], b_st[:], AF.Exp)
                            Ev = E[:].rearrange(
                                "p (lt ij o f) -> p lt ij o f", lt=PLT,
                                ij=NIJ, o=O, f=F)
                            Z = smpool.tile([128, PLT * 36], f32, tag="Z")
                            nc.vector.tensor_reduce(
                                Z[:], Ev.transpose([0, 1, 2, 4, 3]), AX.X,
                                AL.add)
                            Zi = smpool.tile([128, PLT * 36], f32,
                                             tag="Zi")
                            nc.vector.reciprocal(Zi[:], Z[:])
                            Zib = Zi[:].rearrange(
                                "p (lt ij f) -> p lt ij f", lt=PLT,
                                ij=NIJ).unsqueeze(3).broadcast_to(
                                    (128, PLT, NIJ, O, F))
                            nc.vector.tensor_tensor(Ev, Ev, Zib, AL.mult)
                            s_st = smpool.tile([128, PLT * 32], f32,
                                               tag="s")
                            KK = ST_LT * NIJ
                            for half in range(PB):
                                # G = E' * P, one op per half ((lt, ij)
                                # collapses to one affine axis k)
                                G = ghpool.tile([128, ST_LT * 1152], f32,
                                                tag="gh")
                                Gk = G[:].rearrange(
                                    "p (k o u f) -> p k o u f", k=KK, o=O,
                                    u=U, f=F)
                                Pk = P_sts[half][:].rearrange(
                                    "p (k o u f) -> p k o u f", k=KK, o=O,
                                    u=U, f=F)
                                Ek = E[:, half * ST_LT * 144:(half + 1) *
                                       ST_LT * 144].rearrange(
                                    "p (k o f) -> p k o f", k=KK,
                                    o=O).unsqueeze(3).broadcast_to(
                                        (128, KK, O, U, F))
                                nc.vector.tensor_tensor(Gk, Pk, Ek, AL.mult)
                                # s[l, (half, lt, o, u)] = sum_{ij,f} G
                                for lt in range(ST_LT):
                                    glt = half * ST_LT + lt
                                    G5 = G[:, lt * 1152:(lt + 1) *
                                           1152].rearrange(
                                               "p (ij o u f) -> p ij o u f",
                                               ij=NIJ, o=O, u=U, f=F)
                                    nc.vector.tensor_reduce(
                                        s_st[:, glt * 32:(glt + 1) * 32],
                                        G5.transpose([0, 2, 3, 1, 4]),
                                        AX.XY, AL.add)
                            if is_dbg and it == 1:
                                nc.sync.dma_start(dbg["dbg_E1"][:],
                                                  E[:, :ST_LT * 144])
                                nc.sync.dma_start(dbg["dbg_s1"][:],
                                                  s_st[:, :ST_LT * 32])
                            v = squash(s_st, "12")
                        if it < 2:
                            # accumulate logits: b += sum_u P * v
                            dst = b_st if it == 0 else hred
                            for half in range(PB):
                                Hst = ghpool.tile([128, ST_LT * 1152], f32,
                                                  tag="gh")
                                # H = P * v_bcast per lt (v's broadcast AP
                                # needs [ij, (o,u), f] = 3 AP dims; adding
                                # lt would exceed the DVE TENSOR3D limit).
                                for lt in range(ST_LT):
                                    glt = half * ST_LT + lt
                                    H5 = Hst[:, lt * 1152:(lt + 1) *
                                             1152].rearrange(
                                                 "p (ij o u f) -> "
                                                 "p ij o u f",
                                                 ij=NIJ, o=O, u=U, f=F)
                                    vb = v[:, glt * 32:(glt + 1) *
                                           32].rearrange(
                                        "p (o u) -> p o u",
                                        o=O).unsqueeze(1).unsqueeze(
                                            4).broadcast_to(
                                                (128, NIJ, O, U, F))
                                    nc.vector.tensor_tensor(
                                        H5, P5(half, lt), vb, AL.mult)
                                # one segmented reduce over u per half
                                Hk = Hst[:].rearrange(
                                    "p (k o u f) -> p k o u f",
                                    k=ST_LT * NIJ, o=O, u=U, f=F)
                                nc.vector.tensor_reduce(
                                    dst[:, half * ST_LT * 144:(half + 1) *
                                        ST_LT * 144],
                                    Hk.transpose([0, 1, 2, 4, 3]),
                                    AX.X, AL.add)
                            if it == 0 and is_dbg:
                                nc.sync.dma_start(dbg["dbg_b1"][:],
                                                  b_st[:, :ST_LT * 144])
                            if it == 1:
                                nc.vector.tensor_tensor(b_st[:], b_st[:],
                                                        hred[:], AL.add)

                    # v now holds squash(s2): transpose to [32, locs] & stage
                    for glt in range(PLT):
                        r0 = (pr * PLT + glt) * LT_ROWS
                        tp = ptp.tile([32, 128], f32, tag="tp")
                        nc.tensor.transpose(tp[:],
                                            v[:, glt * 32:(glt + 1) * 32],
                                            ident_s[:])
                        nc.scalar.copy(
                            stage[:, r0 * W:r0 * W + LT_ROWS * W], tp[:])

                nc.sync.dma_start(out_d[img], stage[:])

    nc.compile()
    return nc


def _get_compiled():
    global _COMPILED
    if _COMPILED is None:
        _COMPILED = _build()
    return _COMPILED


def _make_consts(weight):
    w = np.asarray(weight, dtype=np.float32)  # [o, f, i, j, u, d]
    wmov = np.zeros((C, NIJ * 128), dtype=np.float32)
    wsum = np.zeros((C, NIJ * 32), dtype=np.float32)
    for o in range(O):
        for f in range(F):
            for ij in range(NIJ):
                i, j = ij // KW, ij % KW
                for u in range(U):
                    for d in range(D):
                        wmov[f * D + d,
                             ij * 128 + o * 32 + u * 4 + f] = w[o, f, i, j,
                                                                u, d]
                        wsum[f * D + d,
                             ij * 32 + o * 8 + u] = 0.25 * w[o, f, i, j, u,
                                                             d]
    return wmov, wsum


def kernel(x, weight):
    x = np.ascontiguousarray(np.asarray(x, dtype=np.float32))
    wmov, wsum = _make_consts(weight)
    ident = np.eye(128, dtype=np.float32)

    nc = _get_compiled()
    in_maps = []
    for c in range(N_CORES):
        xin = x[c * IMG_PER_CORE:(c + 1) * IMG_PER_CORE].reshape(
            IMG_PER_CORE, C, H * W)
        in_maps.append({
            "xin": np.ascontiguousarray(xin),
            "wmov": wmov,
            "wsum": wsum,
            "ident": ident,
        })
    res = bass_utils.run_bass_kernel_spmd(nc, in_maps,
                                          core_ids=list(range(N_CORES)))
    out = np.empty((N_FULL, C, H, W), dtype=np.float32)
    for c in range(N_CORES):
        out[c * IMG_PER_CORE:(c + 1) * IMG_PER_CORE] = res.results[c][
            "out"].reshape(IMG_PER_CORE, C, H, W)
    return out



# revision 7
# speedup vs baseline: 2.2037x; 2.2037x over previous
"""CapsuleConv2d (3-iteration dynamic routing) Bass kernel for 8 TRN2 cores.

Strategy (data-parallel over batch, 2 images per core):
  - priors computed by PE in fp16 (stationary = padded-x window, moving =
    structured weight constants); PSUM fp32.
  - P staged to SBUF twice by ACT (fp16): Pu in (ij,o,f,u) order (u
    innermost) for the H = P*v multiplies, Pf in (o,u,ij,f) order
    ((ij,f) innermost) for the G = E'*P multiplies.  All big DVE
    multiplies run in 2x_1p mode (2-byte dtypes, innermost stride 1).
  - reductions: DVE TensorReduce has no fast mode (1 elem/cycle any
    dtype), so the big contractions are pairwise ADD TREES of packed
    fp16 tensor_tensor ops, which do hit 2x mode: a 9216-col direct
    reduce becomes ~2300 equivalent cycles.  Final tree level outputs
    fp32 (b, s, Z accumulate in fp32 for precision).
  - E = exp(b) is bf16 (fp16 would overflow: b can reach ~30); the
    normalized E' = E/Z is fp16 (values <= 1), G/H/v fp16.
"""
import numpy as np

import concourse.bass as bass
import concourse.bacc as bacc
import concourse.tile as tile
import concourse.mybir as mybir
import concourse.bass_utils as bass_utils

# All ACT functions we use (Exp, Ln, Square, Copy, ...) live together in the
# "natural_log_exp_and_others" table set, but bacc's table-load pass picks a
# per-function set greedily (Ln -> natural_log, Exp -> exp_and_others),
# thrashing ~2.7us table loads between them.  Restrict Exp/Ln to the combined
# set so a single load covers the whole kernel.
_orig_get_tables = bacc.get_activation_tables
_AFT = mybir.ActivationFunctionType


def _patched_get_tables(arch):
    tables = dict(_orig_get_tables(arch))
    for name, funcs in tables.items():
        if name != "natural_log_exp_and_others":
            tables[name] = funcs - {_AFT.Exp, _AFT.Ln}
    return tables


bacc.get_activation_tables = _patched_get_tables

# ---- problem constants (hardcoded; must match setup_inputs) ----
O, F, U, D = 4, 4, 8, 8
KH = KW = 3
NIJ = KH * KW
H = W = 64
C = 32
N_FULL = 16
N_CORES = 8
IMG_PER_CORE = N_FULL // N_CORES
HP, WP = H + 2, W + 2              # padded input
LT_ROWS = 2                        # output rows per 128-loc tile
NLT = H // LT_ROWS                 # 32 loc-tiles per image
ST_LT = 4                          # loc-tiles per super-tile (512 locs)
NST = NLT // ST_LT                 # 8 super-tiles per image
PB = 2                             # super-tiles batched per routing pass
PLT = PB * ST_LT                   # loc-tiles per routing pass (8)
EPS = 1e-12
KK = ST_LT * NIJ                   # collapsed (lt, ij) per half
MH = ST_LT * 32                    # (lt, o, u) per half = 128
BH = ST_LT * 144                   # (lt, ij, o, f) per half = 576

f32 = mybir.dt.float32
f16 = mybir.dt.float16
bf16 = mybir.dt.bfloat16
AL = mybir.AluOpType
AF = mybir.ActivationFunctionType
AX = mybir.AxisListType

_COMPILED = None


def _build(dump=False, repeat=1):
    nc = bacc.Bacc("TRN2", target_bir_lowering=False, debug=False)

    xin_d = nc.dram_tensor("xin", [IMG_PER_CORE, C, H * W], f16,
                           kind="ExternalInput").ap()
    wmov_d = nc.dram_tensor("wmov", [C, NIJ * 128], f16,
                            kind="ExternalInput").ap()
    wsum_d = nc.dram_tensor("wsum", [C, NIJ * 32], f16,
                            kind="ExternalInput").ap()
    ident_d = nc.dram_tensor("ident", [128, 128], f16,
                             kind="ExternalInput").ap()
    out_d = nc.dram_tensor("out", [IMG_PER_CORE, C, H * W], f32,
                           kind="ExternalOutput").ap()

    with tile.TileContext(nc) as tc:
        with tc.tile_pool(name="const", bufs=1) as cpool, \
             tc.tile_pool(name="xpad", bufs=1) as xpool, \
             tc.tile_pool(name="stage", bufs=1) as spool, \
             tc.tile_pool(name="pu", bufs=4) as pupool, \
             tc.tile_pool(name="pf", bufs=4) as pfpool, \
             tc.tile_pool(name="gh", bufs=2) as ghpool, \
             tc.tile_pool(name="small", bufs=2) as smpool, \
             tc.tile_pool(name="tree", bufs=1) as trpool, \
             tc.tile_pool(name="ppri", bufs=2, space="PSUM") as ppri, \
             tc.tile_pool(name="ps0", bufs=1, space="PSUM") as ps0, \
             tc.tile_pool(name="ptp", bufs=1, space="PSUM") as ptp:

            wmov_s = cpool.tile([C, NIJ * 128], f16, tag="wmov")
            wsum_s = cpool.tile([C, NIJ * 32], f16, tag="wsum")
            ident_s = cpool.tile([128, 128], f16, tag="ident")
            eps_s = cpool.tile([128, 1], f32, tag="eps")
            nc.sync.dma_start(wmov_s[:], wmov_d[:])
            nc.sync.dma_start(wsum_s[:], wsum_d[:])
            nc.sync.dma_start(ident_s[:], ident_d[:])
            nc.gpsimd.memset(eps_s[:], EPS)

            for img in range(IMG_PER_CORE):
                xp = xpool.tile([C, HP * WP], f16, tag="xpad")
                nc.gpsimd.memset(xp[:], 0.0)
                xv = xp[:].rearrange("p (h w) -> p h w", h=HP, w=WP)
                nc.sync.dma_start(
                    xv[:, 1:1 + H, 1:1 + W],
                    xin_d[img].rearrange("p (h w) -> p h w", h=H, w=W))
                stage = spool.tile([C, H * W], f32, tag="stage")
                for pr_rep in range((NST // PB) * repeat):
                    pr = pr_rep % (NST // PB)
                    # ---- priors for PB super-tiles (PE, fp16) ----
                    Pu_sts, Pf_sts = [], []
                    s0_st = smpool.tile([128, PLT * 32], f32, tag="s0")
                    for half in range(PB):
                        st = pr * PB + half
                        Pu_st = pupool.tile([128, ST_LT * 1152], f16,
                                            tag="Pu")
                        Pf_st = pfpool.tile([128, ST_LT * 1152], f16,
                                            tag="Pf")
                        Pu_sts.append(Pu_st)
                        Pf_sts.append(Pf_st)
                        for lt in range(ST_LT):
                            r0 = (st * ST_LT + lt) * LT_ROWS
                            glt = half * ST_LT + lt
                            pp = ppri.tile([128, 1152], f32, tag="ppri")
                            s0p = ps0.tile([128, 32], f32, tag="s0p")
                            for ij in range(NIJ):
                                i, j = ij // KW, ij % KW
                                for r in range(LT_ROWS):
                                    xw = xv[:, r0 + i + r, j:j + W]
                                    prow = slice(r * W, (r + 1) * W)
                                    nc.tensor.matmul(
                                        pp[prow, ij * 128:(ij + 1) * 128],
                                        xw,
                                        wmov_s[:, ij * 128:(ij + 1) * 128],
                                        start=True, stop=True)
                                    nc.tensor.matmul(
                                        s0p[prow], xw,
                                        wsum_s[:, ij * 32:(ij + 1) * 32],
                                        start=(ij == 0),
                                        stop=(ij == NIJ - 1))
                            # PSUM pp is (f,o,u) per tap (wmov col
                            # order), so Pu (ij,f,o,u) is a straight copy.
                            nc.scalar.copy(
                                Pu_st[:, lt * 1152:(lt + 1) * 1152], pp[:])
                            # Pf: (o,u,ij,f); one ACT copy per o keeps the
                            # transposed access pattern at 3 free dims.
                            pfs = Pf_st[:, lt * 1152:(lt + 1) * 1152]
                            ppv = pp[:].rearrange(
                                "p (ij f o u) -> p ij f o u", ij=NIJ, f=F,
                                o=O, u=U)
                            pfv = pfs.rearrange(
                                "p (o u ij f) -> p o u ij f", o=O, u=U,
                                ij=NIJ, f=F)
                            for o_ in range(O):
                                nc.scalar.copy(
                                    pfv[:, o_].transpose([0, 1, 2, 3]),
                                    ppv[:, :, :, o_].transpose([0, 3, 1, 2]))
                            nc.scalar.copy(
                                s0_st[:, glt * 32:(glt + 1) * 32], s0p[:])

                    # ------- routing on this super-tile pair -------
                    def squash(s_st, tagp):
                        # s_st: [128, (lt, o, u)] fp32; returns v fp16
                        sq = smpool.tile([128, PLT * 32], f32,
                                         tag=f"sq{tagp}")
                        nc.scalar.activation(sq[:], s_st[:], AF.Square)
                        n2 = smpool.tile([128, PLT * O], f32,
                                         tag=f"n2{tagp}")
                        nc.vector.tensor_reduce(
                            n2[:],
                            sq[:].rearrange("p (lt o u) -> p lt o u",
                                            lt=PLT, o=O, u=U),
                            AX.X, AL.add)
                        Ltile = smpool.tile([128, PLT * O], f32,
                                            tag=f"L{tagp}")
                        nc.scalar.activation(Ltile[:], n2[:], AF.Ln,
                                             bias=eps_s[:])
                        t_ = smpool.tile([128, PLT * O], f32,
                                         tag=f"t{tagp}")
                        nc.scalar.activation(t_[:], Ltile[:], AF.Exp,
                                             scale=0.5)
                        r5 = smpool.tile([128, PLT * O], f32,
                                         tag=f"r5{tagp}")
                        nc.vector.reciprocal(r5[:], t_[:])
                        xr = smpool.tile([128, PLT * O], f32,
                                         tag=f"xr{tagp}")
                        nc.vector.scalar_tensor_tensor(
                            xr[:], r5[:], 0.5, n2[:], AL.mult, AL.mult)
                        nc.vector.scalar_tensor_tensor(
                            t_[:], t_[:], 0.5, xr[:], AL.mult, AL.add)
                        pw = smpool.tile([128, PLT * O], f32,
                                         tag=f"pw{tagp}")
                        nc.vector.scalar_tensor_tensor(
                            pw[:], n2[:], 1.0, t_[:], AL.add, AL.mult)
                        rw = smpool.tile([128, PLT * O], f32,
                                         tag=f"rw{tagp}")
                        nc.vector.reciprocal(rw[:], pw[:])
                        fi = smpool.tile([128, PLT * O], f32,
                                         tag=f"fi{tagp}")
                        nc.vector.tensor_tensor(fi[:], n2[:], rw[:], AL.mult)
                        v = smpool.tile([128, PLT * 32], f16,
                                        tag=f"v{tagp}")
                        fib = fi[:].rearrange("p (lt o) -> p lt o",
                                              lt=PLT).unsqueeze(3)
                        nc.vector.tensor_tensor(
                            v[:].rearrange("p (lt o u) -> p lt o u",
                                           lt=PLT, o=O, u=U),
                            s_st[:].rearrange("p (lt o u) -> p lt o u",
                                              lt=PLT, o=O, u=U),
                            fib.broadcast_to((128, PLT, O, U)), AL.mult)
                        return v

                    v = squash(s0_st, "0")

                    b_st = smpool.tile([128, PLT * 144], f32, tag="b")
                    hred = smpool.tile([128, PLT * 144], f32, tag="hred")
                    for it in range(3):
                        if it > 0:
                            # E = exp(b) bf16; Z = sum_o E (add tree);
                            # E' = E * (1/Z) -> fp16
                            # E = exp(b): b is (lt,ij,f,o); write E
                            # as (lt,o,ij,f) per loc-tile (3-dim APs)
                            E = smpool.tile([128, PLT * 144], bf16, tag="E")
                            for glt in range(PLT):
                                bl = b_st[:, glt * 144:(glt + 1) *
                                          144].rearrange(
                                    "p (ij f o) -> p ij f o", ij=NIJ, f=F,
                                    o=O)
                                el = E[:, glt * 144:(glt + 1) *
                                       144].rearrange(
                                    "p (o ij f) -> p o ij f", o=O, ij=NIJ,
                                    f=F)
                                nc.scalar.activation(
                                    el, bl.transpose([0, 3, 1, 2]), AF.Exp)
                            # Z tree: sum over o (stride 36 per lt)
                            E2 = E[:].rearrange("p (lt t) -> p lt t",
                                                lt=PLT, t=144)
                            zt1 = trpool.tile([128, PLT * 72], bf16,
                                              tag="zt1")
                            zt1v = zt1[:].rearrange("p (lt t) -> p lt t",
                                                    lt=PLT, t=72)
                            nc.vector.tensor_tensor(
                                zt1v, E2[:, :, 0:72], E2[:, :, 72:144],
                                AL.add)
                            Z = smpool.tile([128, PLT * 36], f32, tag="Z")
                            nc.vector.tensor_tensor(
                                Z[:].rearrange("p (lt t) -> p lt t",
                                               lt=PLT, t=36),
                                zt1v[:, :, 0:36], zt1v[:, :, 36:72],
                                AL.add)
                            Zi = smpool.tile([128, PLT * 36], f32,
                                             tag="Zi")
                            nc.vector.reciprocal(Zi[:], Z[:])
                            Zi16 = smpool.tile([128, PLT * 36], bf16,
                                               tag="Zi16")
                            nc.scalar.copy(Zi16[:], Zi[:])
                            # E' = E * (1/Z): all (lt,o,ij,f), 2x mode
                            Ep = smpool.tile([128, PLT * 144], f16,
                                             tag="Ep")
                            Zib = Zi16[:].rearrange(
                                "p (lt t) -> p lt t",
                                lt=PLT).unsqueeze(2).broadcast_to(
                                    (128, PLT, O, 36))
                            nc.vector.tensor_tensor(
                                Ep[:].rearrange("p (lt o t) -> p lt o t",
                                                lt=PLT, o=O, t=36),
                                E[:].rearrange("p (lt o t) -> p lt o t",
                                               lt=PLT, o=O, t=36),
                                Zib, AL.mult)
                            s_st = smpool.tile([128, PLT * 32], f32,
                                               tag="s")
                            for half in range(PB):
                                # G = E' * Pf (fp16, 2x), per loc-tile
                                G = ghpool.tile([128, ST_LT * 1152], f16,
                                                tag="gg")
                                for lt in range(ST_LT):
                                    glt = half * ST_LT + lt
                                    Gv = G[:, lt * 1152:(lt + 1) *
                                           1152].rearrange(
                                        "p (o u t) -> p o u t", o=O, u=U,
                                        t=36)
                                    Pfv = Pf_sts[half][:, lt * 1152:
                                                       (lt + 1) *
                                                       1152].rearrange(
                                        "p (o u t) -> p o u t", o=O, u=U,
                                        t=36)
                                    Eb = Ep[:, glt * 144:(glt + 1) *
                                            144].rearrange(
                                        "p (o t) -> p o t",
                                        o=O).unsqueeze(2).broadcast_to(
                                            (128, O, U, 36))
                                    nc.vector.tensor_tensor(Gv, Pfv, Eb,
                                                            AL.mult)
                                # s tree over (ij,f)=36 per (lt,o,u)
                                Gt = G[:].rearrange("p (M t) -> p M t",
                                                    M=MH, t=36)
                                st1 = trpool.tile([128, MH * 18], f16,
                                                  tag="st1")
                                s1v = st1[:].rearrange(
                                    "p (M t) -> p M t", M=MH, t=18)
                                nc.vector.tensor_tensor(
                                    s1v, Gt[:, :, 0:18], Gt[:, :, 18:36],
                                    AL.add)
                                st2 = trpool.tile([128, MH * 9], f16,
                                                  tag="st2")
                                s2v = st2[:].rearrange(
                                    "p (M t) -> p M t", M=MH, t=9)
                                nc.vector.tensor_tensor(
                                    s2v, s1v[:, :, 0:9], s1v[:, :, 9:18],
                                    AL.add)
                                st3 = trpool.tile([128, MH * 4], f16,
                                                  tag="st3")
                                s3v = st3[:].rearrange(
                                    "p (M t) -> p M t", M=MH, t=4)
                                nc.vector.tensor_tensor(
                                    s3v, s2v[:, :, 0:4], s2v[:, :, 4:8],
                                    AL.add)
                                st4 = trpool.tile([128, MH * 2], f16,
                                                  tag="st4")
                                s4v = st4[:].rearrange(
                                    "p (M t) -> p M t", M=MH, t=2)
                                nc.vector.tensor_tensor(
                                    s4v, s3v[:, :, 0:2], s3v[:, :, 2:4],
                                    AL.add)
                                st5 = trpool.tile([128, MH], f16,
                                                  tag="st5")
                                nc.vector.tensor_tensor(
                                    st5[:], s4v[:, :, 0], s4v[:, :, 1],
                                    AL.add)
                                # + carry (ij,f idx 8 of 0..8 nines)
                                nc.vector.tensor_tensor(
                                    s_st[:, half * MH:(half + 1) * MH],
                                    st5[:], s2v[:, :, 8], AL.add)
                            v = squash(s_st, "12")
                        if it < 2:
                            # b += sum_u Pu * v   (H fp16 2x; add tree)
                            dst = b_st if it == 0 else hred
                            for half in range(PB):
                                Hst = ghpool.tile([128, ST_LT * 1152], f16,
                                                  tag="hh")
                                for lt in range(ST_LT):
                                    glt = half * ST_LT + lt
                                    H5 = Hst[:, lt * 1152:(lt + 1) *
                                             1152].rearrange(
                                        "p (k t) -> p k t", k=NIJ * F,
                                        t=32)
                                    Pu5 = Pu_sts[half][:, lt * 1152:
                                                       (lt + 1) *
                                                       1152].rearrange(
                                        "p (k t) -> p k t", k=NIJ * F,
                                        t=32)
                                    vb = v[:, glt * 32:(glt + 1) *
                                           32].unsqueeze(1).broadcast_to(
                                        (128, NIJ * F, 32))
                                    nc.vector.tensor_tensor(
                                        H5, Pu5, vb, AL.mult)
                                Hk = Hst[:].rearrange(
                                    "p (m u) -> p m u", m=BH, u=U)
                                bt1 = trpool.tile([128, BH * 4], f16,
                                                  tag="bt1")
                                b1v = bt1[:].rearrange(
                                    "p (m t) -> p m t", m=BH, t=4)
                                nc.vector.tensor_tensor(
                                    b1v, Hk[:, :, 0:4], Hk[:, :, 4:8],
                                    AL.add)
                                bt2 = trpool.tile([128, BH * 2], f16,
                                                  tag="bt2")
                                b2v = bt2[:].rearrange(
                                    "p (m t) -> p m t", m=BH, t=2)
                                nc.vector.tensor_tensor(
                                    b2v, b1v[:, :, 0:2], b1v[:, :, 2:4],
                                    AL.add)
                                nc.vector.tensor_tensor(
                                    dst[:, half * BH:(half + 1) * BH],
                                    b2v[:, :, 0], b2v[:, :, 1], AL.add)
                            if it == 1:
                                nc.vector.tensor_tensor(b_st[:], b_st[:],
                                                        hred[:], AL.add)

                    # v (fp16) -> transpose to [32, locs] & stage fp32
                    for glt in range(PLT):
                        r0 = (pr * PLT + glt) * LT_ROWS
                        tp = ptp.tile([32, 128], f16, tag="tp")
                        nc.tensor.transpose(tp[:],
                                            v[:, glt * 32:(glt + 1) * 32],
                                            ident_s[:])
                        nc.scalar.copy(
                            stage[:, r0 * W:r0 * W + LT_ROWS * W], tp[:])

                nc.sync.dma_start(out_d[img], stage[:])

    nc.compile()
    return nc


def _get_compiled():
    global _COMPILED
    if _COMPILED is None:
        _COMPILED = _build()
    return _COMPILED


def _make_consts(weight):
    w = np.asarray(weight, dtype=np.float32)  # [o, f, i, j, u, d]
    wmov = np.zeros((C, NIJ * 128), dtype=np.float16)
    wsum = np.zeros((C, NIJ * 32), dtype=np.float16)
    for o in range(O):
        for f in range(F):
            for ij in range(NIJ):
                i, j = ij // KW, ij % KW
                for u in range(U):
                    for d in range(D):
                        wmov[f * D + d,
                             ij * 128 + f * 32 + o * 8 + u] = w[o, f, i, j,
                                                                u, d]
                        wsum[f * D + d,
                             ij * 32 + o * 8 + u] = 0.25 * w[o, f, i, j, u,
                                                             d]
    return wmov, wsum


def make_in_maps(x, weight):
    x16 = np.asarray(x).astype(np.float16)
    wmov, wsum = _make_consts(weight)
    ident = np.eye(128, dtype=np.float16)
    in_maps = []
    for c in range(N_CORES):
        xin = x16[c * IMG_PER_CORE:(c + 1) * IMG_PER_CORE].reshape(
            IMG_PER_CORE, C, H * W)
        in_maps.append({
            "xin": np.ascontiguousarray(xin),
            "wmov": wmov,
            "wsum": wsum,
            "ident": ident,
        })
    return in_maps


def kernel(x, weight):
    nc = _get_compiled()
    in_maps = make_in_maps(x, weight)
    res = bass_utils.run_bass_kernel_spmd(nc, in_maps,
                                          core_ids=list(range(N_CORES)))
    out = np.empty((N_FULL, C, H, W), dtype=np.float32)
    for c in range(N_CORES):
        out[c * IMG_PER_CORE:(c + 1) * IMG_PER_CORE] = res.results[c][
            "out"].reshape(IMG_PER_CORE, C, H, W)
    return out
